# revision 1
# baseline (speedup 1.0000x reference)
"""Trainium2 Bass kernel for nn_Decoder (pre-LN transformer decoder layer).

Sharding: 8 cores = 4 batches x 2 sequence-halves. Core pid -> (batch=pid//2,
s=pid%2). s=0 handles query tokens [0,T0), s=1 handles [T0,L). Each core
computes k/v for its key range on its own (s=1 recomputes the prefix
projections), so no collectives are needed; the host concatenates outputs.

Layout strategy: activations token-major for LN/residual (per-partition
scalars via bn_stats), feature-major (transposed via bf16 PE transpose) for
the matmul chain. Attention computed fully on-chip flash-style:
scoresT [keys, q] -> exp (ACT, scale=1/HD) -> PV matmul with 64 ones-columns
(shared tail block addressed via a strided AP) so the softmax denominator
lands in PSUM partitions 64:127. All matmuls bf16 (weights cast during DMA),
everything else fp32.
"""
import os
import sys

sys.path.insert(0, "/opt/trn_rl_repo")

import contextlib

import numpy as np

import concourse.bass as bass
import concourse.mybir as mybir
import concourse.tile as tile
import concourse.tile_utils as tile_utils
from concourse import bacc
from concourse.bass_utils import run_bass_kernel_spmd
from concourse.masks import make_identity

# trn2 has 224KB/partition physical, ~208 usable; default cap is stale 192.
tile_utils.max_sbuf_usage = 206 * 1024

F32 = mybir.dt.float32
BF16 = mybir.dt.bfloat16
AF = mybir.ActivationFunctionType
ALU = mybir.AluOpType

if os.environ.get("DECODER_DIMS"):
    B, L, D, H, I, T0 = (int(v) for v in os.environ["DECODER_DIMS"].split(","))
else:
    B, L, D, H, I, T0 = 4, 2048, 768, 12, 3072, 1280
HD = 64
T1 = L - T0
EPS = 1e-5
N_CORES = 2 * B
ND = D // 128
NI = I // 128
NH = H
MASK_BIG = -1e9
BN_SUB = 256


def attn_spans(q_start, q_len, span=512):
    out = []
    q0 = q_start
    while q0 < q_start + q_len:
        w = min(span, q_start + q_len - q0)
        out.append((q0, w, q0 // 128))
        q0 += w
    return out


def build_body(nc, tc, ctx, io, q_start, q_len, kv_len):
    x, w_qkv, w_o, w1, w2, bqkv, bo, b1, b2, g1, bl1, g2, bl2, out = io
    NT_KV = kv_len // 128
    NT_Q = q_len // 128

    # ---------------- constant tiles ----------------
    consts = ctx.enter_context(tc.tile_pool(name="consts", bufs=1))
    ident = consts.tile([128, 128], BF16, tag="ident")
    make_identity(nc, ident[:])
    causal = consts.tile([128, 128], F32, tag="causal")
    nc.vector.memset(causal[:], 0.0)
    nc.gpsimd.affine_select(
        out=causal[:], in_=causal[:], pattern=[[1, 128]],
        channel_multiplier=-1, base=0, compare_op=ALU.is_ge, fill=MASK_BIG)
    eps_t = consts.tile([128, 1], F32, tag="eps")
    nc.vector.memset(eps_t[:], EPS)

    def bcast(vec_ap, n, name, dtype=F32):
        t = consts.tile([128, n], dtype, tag=name)
        src = bass.AP(tensor=vec_ap.tensor, offset=vec_ap.offset,
                      ap=[[0, 128]] + vec_ap.ap)
        nc.gpsimd.dma_start(out=t[:], in_=src)
        return t

    g1_bc = bcast(g1, D, "g1_bc", BF16)
    bl1_bc = bcast(bl1, D, "bl1_bc", BF16)
    g2_bc = bcast(g2, D, "g2_bc", BF16)
    bl2_bc = bcast(bl2, D, "bl2_bc", BF16)
    bo_bc = bcast(bo, D, "bo_bc")          # f32: residual path
    b2_bc = bcast(b2, D, "b2_bc")          # f32: pre-gelu
    bv_src = bass.AP(tensor=bqkv.tensor, offset=bqkv.offset + 2 * HD,
                     ap=[[0, 128], [3 * HD, NH], [1, HD]])
    bv_bc = consts.tile([128, NH * HD], F32, tag="bv_bc")
    nc.gpsimd.dma_start(out=bv_bc[:], in_=bv_src)

    # Long-lived pools; stack (open order) must be reverse of close order:
    # closes: ht (after B), qkv (after C), wearly (after C), rest at end.
    oa_pool = ctx.enter_context(tc.tile_pool(name="oa_pool", bufs=5))
    h2_pool = ctx.enter_context(tc.tile_pool(name="h2_pool", bufs=2))
    at_cm = tc.tile_pool(name="attn_pool", bufs=1)
    at_pool = at_cm.__enter__()
    wearly_cm = tc.tile_pool(name="wearly", bufs=1)
    wearly = wearly_cm.__enter__()

    # ---------------- early weights (qkv) ----------------
    wqk = wearly.tile([128, ND, NH * 128], BF16, tag="wqk")
    for d in range(ND):
        src = bass.AP(tensor=w_qkv.tensor,
                      offset=w_qkv.offset + d * 128 * 3 * D,
                      ap=[[3 * D, 128], [3 * HD, NH], [1, 2 * HD]])
        nc.gpsimd.dma_start(out=wqk[:, d, :], in_=src)
    wv = wearly.tile([128, ND, NH * HD], BF16, tag="wv")
    for d in range(ND):
        src = bass.AP(tensor=w_qkv.tensor,
                      offset=w_qkv.offset + d * 128 * 3 * D + 2 * HD,
                      ap=[[3 * D, 128], [3 * HD, NH], [1, HD]])
        nc.gpsimd.dma_start(out=wv[:, d, :], in_=src)
    bqk = wearly.tile([128, NH], F32, tag="bqk")
    nc.gpsimd.dma_start(
        out=bqk[:],
        in_=bass.AP(tensor=bqkv.tensor, offset=bqkv.offset,
                    ap=[[1, 128], [3 * HD, NH]]))
    qkv_cm = tc.tile_pool(name="qkv_pool", bufs=1)
    qkv_pool = qkv_cm.__enter__()
    ht_cm = tc.tile_pool(name="ht_pool", bufs=1)
    ht_pool = ht_cm.__enter__()

    # ---------------- phase A: x -> LN1 -> h (bf16) -> hT ----------------
    hT = ht_pool.tile([128, ND, kv_len], BF16, tag="hT")

    def layernorm_tokmajor(x_t, g_bc, b_bc, pool, tagp):
        stats = pool.tile([128, D // BN_SUB, 6], F32, tag=tagp + "_stats")
        xs = x_t[:].rearrange("p (s c) -> p s c", c=BN_SUB)
        for sgi in range(D // BN_SUB):
            nc.vector.bn_stats(out=stats[:, sgi, :], in_=xs[:, sgi, :])
        mv = pool.tile([128, 2], F32, tag=tagp + "_mv")
        nc.vector.bn_aggr(out=mv[:], in_=stats[:])
        rstd = pool.tile([128, 1], F32, tag=tagp + "_rstd")
        nc.scalar.activation(out=rstd[:], in_=mv[:, 1:2],
                             func=AF.Sqrt, bias=eps_t[:], scale=1.0)
        nc.vector.reciprocal(out=rstd[:], in_=rstd[:])
        hc = pool.tile([128, D], BF16, tag=tagp + "_hc")
        nc.vector.tensor_scalar(
            out=hc[:], in0=x_t, scalar1=mv[:, 0:1], scalar2=rstd[:],
            op0=ALU.subtract, op1=ALU.mult)
        nc.vector.tensor_tensor(out=hc[:], in0=hc[:], in1=g_bc[:],
                                op=ALU.mult)
        hb = pool.tile([128, D], BF16, tag=tagp + "_hb")
        nc.vector.tensor_tensor(out=hb[:], in0=hc[:], in1=b_bc[:],
                                op=ALU.add)
        return hb

    with contextlib.ExitStack() as phA:
        xpool = phA.enter_context(tc.tile_pool(name="xpool", bufs=3))
        lnp = phA.enter_context(tc.tile_pool(name="lnp", bufs=2))
        tpp = phA.enter_context(
            tc.tile_pool(name="tpp", bufs=4, space="PSUM"))
        for tw in range(NT_KV):
            x_t = xpool.tile([128, D], F32, tag="x_t")
            nc.sync.dma_start(out=x_t[:], in_=x[tw * 128:(tw + 1) * 128, :])
            hb = layernorm_tokmajor(x_t[:], g1_bc, bl1_bc, lnp, "ln1")
            for d in range(ND):
                pt = tpp.tile([128, 128], BF16, tag="tp_ps")
                nc.tensor.transpose(pt[:], hb[:, d * 128:(d + 1) * 128],
                                    ident[:])
                nc.vector.tensor_copy(
                    out=hT[:, d, tw * 128:(tw + 1) * 128], in_=pt[:])

    # ---------------- phase B: qkv projections ----------------
    # head h at partition half 64*(h%2), pair h//2, in both qT2 and kT2.
    qT2 = qkv_pool.tile([128, NH // 2, q_len], BF16, tag="qT2")
    kT2 = qkv_pool.tile([128, NH // 2, kv_len], BF16, tag="kT2")
    # v token-major: per (tok-window, head) a [128, 128] block of
    # [v (64 cols) | ones (64 cols)] so the PV matmul also produces the
    # softmax denominator on PSUM partitions 64:127.
    vaug = qkv_pool.tile([128, NT_KV, NH, 128], BF16, tag="vaug")

    def vaug_lhsT(kt, h):
        return vaug[:, kt, h, :]

    with contextlib.ExitStack() as phB:
        qkps = phB.enter_context(
            tc.tile_pool(name="qkps", bufs=2, space="PSUM"))
        vps = phB.enter_context(tc.tile_pool(name="vps", bufs=2, space="PSUM"))

        for sp0 in range(0, kv_len, 512):
            w = min(512, kv_len - sp0)
            for h in range(NH):
                hb2, hp = 64 * (h % 2), h // 2
                pq = qkps.tile([128, 512], F32, tag="pqk")
                for d in range(ND):
                    nc.tensor.matmul(
                        pq[:, 0:w], wqk[:, d, h * 128:(h + 1) * 128],
                        hT[:, d, sp0:sp0 + w],
                        start=(d == 0), stop=(d == ND - 1))
                nc.vector.tensor_scalar_add(
                    out=kT2[hb2:hb2 + 64, hp, sp0:sp0 + w],
                    in0=pq[64:128, 0:w], scalar1=bqk[64:128, h:h + 1])
                lo = max(sp0, q_start)
                hi = min(sp0 + w, q_start + q_len)
                if lo < hi:
                    nc.vector.tensor_scalar_add(
                        out=qT2[hb2:hb2 + 64, hp, lo - q_start:hi - q_start],
                        in0=pq[0:64, lo - sp0:hi - sp0],
                        scalar1=bqk[0:64, h:h + 1])
        half = NH * HD // 2
        nhh = NH // 2
        for tw in range(NT_KV):
            for hf in range(2):
                pv = vps.tile([128, half], F32, tag="pv")
                for d in range(ND):
                    nc.tensor.matmul(
                        pv[:], hT[:, d, tw * 128:(tw + 1) * 128],
                        wv[:, d, hf * half:(hf + 1) * half],
                        start=(d == 0), stop=(d == ND - 1))
                dst = vaug[:, tw, hf * nhh:(hf + 1) * nhh, 0:HD]
                bvs = bv_bc[:, hf * half:(hf + 1) * half].rearrange(
                    "p (h c) -> p h c", c=HD)
                nc.vector.tensor_tensor(
                    out=dst, in0=pv[:].rearrange("p (h c) -> p h c", c=HD),
                    in1=bvs, op=ALU.add)
            nc.vector.memset(vaug[:, tw, :, HD:128], 1.0)

    ht_cm.__exit__(None, None, None)

    # ---------------- phase C: attention ----------------
    attnT = at_pool.tile([128, ND, q_len], BF16, tag="attnT")

    with contextlib.ExitStack() as phC:
        sps = phC.enter_context(tc.tile_pool(name="sps", bufs=3, space="PSUM"))
        ops_ = phC.enter_context(
            tc.tile_pool(name="ops", bufs=2, space="PSUM"))
        epool = phC.enter_context(tc.tile_pool(name="epool", bufs=4))
        rpool = phC.enter_context(tc.tile_pool(name="rpool", bufs=2))

        for h in range(NH):
            hb2, hp = 64 * (h % 2), h // 2
            for (q0, w, nfull) in attn_spans(q_start, q_len):
                po = ops_.tile([128, 512], F32, tag="po")
                ndiag = w // 128
                ktot = nfull + ndiag

                def scores_mm(ps_slice, kt, c0):
                    nc.tensor.matmul(
                        ps_slice,
                        kT2[hb2:hb2 + 64, hp, kt * 128:(kt + 1) * 128],
                        qT2[hb2:hb2 + 64, hp,
                            q0 + c0 - q_start:q0 + w - q_start],
                        start=True, stop=True)

                def pv_mm(kt, et_slice, c0):
                    nc.tensor.matmul(
                        po[:, c0:w], vaug[:, kt, h, :], et_slice,
                        start=(kt == 0), stop=(kt == ktot - 1))

                # full key-tiles, two per PSUM bank (bf16 scores) so each
                # ACT exp covers 2*w elements (amortizes the 352-cyc fixed
                # ACTIVATE overhead)
                kt = 0
                while kt < nfull:
                    npair = min(2, nfull - kt)
                    ps = sps.tile([128, 2, 512], F32, tag="ps")
                    for jj in range(npair):
                        scores_mm(ps[:, jj, 0:w], kt + jj, 0)
                    et = epool.tile([128, 2, 512], BF16, tag="et")
                    nc.scalar.activation(out=et[:, 0:npair, 0:w],
                                         in_=ps[:, 0:npair, 0:w],
                                         func=AF.Exp, bias=0.0, scale=1.0 / HD)
                    for jj in range(npair):
                        pv_mm(kt + jj, et[:, jj, 0:w], 0)
                    kt += npair
                # diagonal key-tiles: causal mask added on the first 128
                # query columns before exp
                for diag_j in range(ndiag):
                    kt = nfull + diag_j
                    c0 = 128 * diag_j
                    wj = w - c0
                    ps = sps.tile([128, 2, 512], F32, tag="ps")
                    scores_mm(ps[:, 0, 0:wj], kt, c0)
                    nc.vector.tensor_tensor(
                        out=ps[:, 0, 0:128], in0=ps[:, 0, 0:128],
                        in1=causal[:], op=ALU.add)
                    et = epool.tile([128, 2, 512], BF16, tag="et")
                    nc.scalar.activation(out=et[:, 0, 0:wj], in_=ps[:, 0, 0:wj],
                                         func=AF.Exp, bias=0.0, scale=1.0 / HD)
                    pv_mm(kt, et[:, 0, 0:wj], c0)
                # 1/den = (1/sqrt(den))^2: rsqrt on ACT (table op), square
                # folded into the two cheap DVE multiplies; iterative DVE
                # reciprocal costs ~6.6 cycles/elem.
                rt = rpool.tile([64, 512], F32, tag="rt")
                nc.scalar.activation(out=rt[:, 0:w], in_=po[64:128, 0:w],
                                     func=AF.Abs_reciprocal_sqrt,
                                     bias=0.0, scale=1.0)
                rt2 = rpool.tile([64, 512], F32, tag="rt2")
                nc.vector.tensor_tensor(out=rt2[:, 0:w], in0=rt[:, 0:w],
                                        in1=rt[:, 0:w], op=ALU.mult)
                nc.vector.tensor_tensor(
                    out=attnT[64 * (h % 2):64 * (h % 2) + 64, hp,
                              q0 - q_start:q0 - q_start + w],
                    in0=po[0:64, 0:w], in1=rt2[:, 0:w], op=ALU.mult)

    qkv_cm.__exit__(None, None, None)
    wearly_cm.__exit__(None, None, None)

    # ------------- phases D+E interleaved per 512-token chunk -------------
    wffn_cm = tc.tile_pool(name="wffn", bufs=1)
    wffn = wffn_cm.__enter__()
    wo_sb = wffn.tile([128, ND, D], BF16, tag="wo_sb")
    for a in range(ND):
        nc.gpsimd.dma_start(out=wo_sb[:, a, :],
                            in_=w_o[a * 128:(a + 1) * 128, :])
    w1_sb = wffn.tile([128, ND, I], BF16, tag="w1_sb")
    for d in range(ND):
        nc.gpsimd.dma_start(out=w1_sb[:, d, :],
                            in_=w1[d * 128:(d + 1) * 128, :])
    w2_sb = wffn.tile([128, NI, D], BF16, tag="w2_sb")
    for i_ in range(NI):
        nc.gpsimd.dma_start(out=w2_sb[:, i_, :],
                            in_=w2[i_ * 128:(i_ + 1) * 128, :])
    b1_sb = wffn.tile([128, NI], F32, tag="b1_sb")
    nc.gpsimd.dma_start(
        out=b1_sb[:],
        in_=bass.AP(tensor=b1.tensor, offset=b1.offset,
                    ap=[[1, 128], [128, NI]]))

    with contextlib.ExitStack() as phDE:
        wops = phDE.enter_context(
            tc.tile_pool(name="wops", bufs=2, space="PSUM"))
        tpp2 = phDE.enter_context(
            tc.tile_pool(name="tpp2", bufs=2, space="PSUM"))
        xpool2 = phDE.enter_context(tc.tile_pool(name="xpool2", bufs=2))
        lnp2 = phDE.enter_context(tc.tile_pool(name="lnp2", bufs=2))
        f1ps = phDE.enter_context(
            tc.tile_pool(name="f1ps", bufs=2, space="PSUM"))
        f2ps = phDE.enter_context(
            tc.tile_pool(name="f2ps", bufs=2, space="PSUM"))
        opool = phDE.enter_context(tc.tile_pool(name="opool", bufs=2))
        Dh = D // 2

        for c0 in range(0, q_len, 512):
            cw = min(512, q_len - c0)
            ctws = cw // 128
            h2T = h2_pool.tile([128, ND, 512], BF16, tag="h2T")
            oa_tiles = []
            # --- phase D for this chunk: w_o + residual + LN2 + transpose
            for twl in range(ctws):
                tw = c0 // 128 + twl
                xo = xpool2.tile([128, D], F32, tag="xo")
                nc.sync.dma_start(
                    out=xo[:],
                    in_=x[q_start + tw * 128:q_start + (tw + 1) * 128, :])
                nc.vector.tensor_tensor(out=xo[:], in0=xo[:], in1=bo_bc[:],
                                        op=ALU.add)
                oa = oa_pool.tile([128, D], F32, tag="oa")
                oa_tiles.append(oa)
                for hf in range(2):
                    pw = wops.tile([128, Dh], F32, tag="pw")
                    for a in range(ND):
                        nc.tensor.matmul(
                            pw[:], attnT[:, a, tw * 128:(tw + 1) * 128],
                            wo_sb[:, a, hf * Dh:(hf + 1) * Dh],
                            start=(a == 0), stop=(a == ND - 1))
                    nc.vector.tensor_tensor(
                        out=oa[:, hf * Dh:(hf + 1) * Dh], in0=pw[:],
                        in1=xo[:, hf * Dh:(hf + 1) * Dh], op=ALU.add)
                hb = layernorm_tokmajor(oa[:], g2_bc, bl2_bc, lnp2, "ln2")
                for d in range(ND):
                    pt = tpp2.tile([128, 128], BF16, tag="tp2_ps")
                    nc.tensor.transpose(pt[:], hb[:, d * 128:(d + 1) * 128],
                                        ident[:])
                    nc.vector.tensor_copy(
                        out=h2T[:, d, twl * 128:(twl + 1) * 128], in_=pt[:])
            # --- phase E for this chunk: FFN + gelu + residual -> out
            ff1 = wffn.tile([128, NI, 512], BF16, tag="ff1")
            for i_ in range(NI):
                pf = f1ps.tile([128, 512], F32, tag="pf1")
                for d in range(ND):
                    nc.tensor.matmul(
                        pf[:, 0:cw], w1_sb[:, d, i_ * 128:(i_ + 1) * 128],
                        h2T[:, d, 0:cw],
                        start=(d == 0), stop=(d == ND - 1))
                nc.vector.tensor_scalar_add(
                    out=ff1[:, i_, 0:cw], in0=pf[:, 0:cw],
                    scalar1=b1_sb[:, i_:i_ + 1])
            for twl in range(ctws):
                tw = c0 // 128 + twl
                ot = opool.tile([128, D], F32, tag="ot")
                for hf in range(2):
                    pg = f2ps.tile([128, Dh], F32, tag="pf2")
                    for i_ in range(NI):
                        nc.tensor.matmul(
                            pg[:], ff1[:, i_, twl * 128:(twl + 1) * 128],
                            w2_sb[:, i_, hf * Dh:(hf + 1) * Dh],
                            start=(i_ == 0), stop=(i_ == NI - 1))
                    sl = slice(hf * Dh, (hf + 1) * Dh)
                    gb = opool.tile([128, Dh], F32, tag="gb")
                    nc.vector.tensor_tensor(out=gb[:], in0=pg[:],
                                            in1=b2_bc[:, sl], op=ALU.add)
                    nc.scalar.activation(out=gb[:], in_=gb[:], func=AF.Gelu,
                                         bias=0.0, scale=1.0)
                    nc.vector.tensor_tensor(out=ot[:, sl], in0=gb[:],
                                            in1=oa_tiles[twl][:, sl],
                                            op=ALU.add)
                nc.sync.dma_start(out=out[tw * 128:(tw + 1) * 128, :],
                                  in_=ot[:])

    wffn_cm.__exit__(None, None, None)
    at_cm.__exit__(None, None, None)


_NC_CACHE = {}


def build_kernel():
    key = (B, L, D, H, I, T0)
    if key in _NC_CACHE:
        return _NC_CACHE[key]
    nc = bacc.Bacc("TRN2", target_bir_lowering=False, debug=False,
                   num_devices=N_CORES)
    x = nc.dram_tensor("x", [L, D], F32, kind="ExternalInput").ap()
    w_qkv = nc.dram_tensor("w_qkv", [D, 3 * D], F32, kind="ExternalInput").ap()
    w_o = nc.dram_tensor("w_o", [D, D], F32, kind="ExternalInput").ap()
    w1 = nc.dram_tensor("w1", [D, I], F32, kind="ExternalInput").ap()
    w2 = nc.dram_tensor("w2", [I, D], F32, kind="ExternalInput").ap()
    bqkv = nc.dram_tensor("b_qkv", [3 * D], F32, kind="ExternalInput").ap()
    bo = nc.dram_tensor("b_o", [D], F32, kind="ExternalInput").ap()
    b1 = nc.dram_tensor("b1", [I], F32, kind="ExternalInput").ap()
    b2 = nc.dram_tensor("b2", [D], F32, kind="ExternalInput").ap()
    g1 = nc.dram_tensor("ln1_g", [D], F32, kind="ExternalInput").ap()
    bl1 = nc.dram_tensor("ln1_b", [D], F32, kind="ExternalInput").ap()
    g2 = nc.dram_tensor("ln2_g", [D], F32, kind="ExternalInput").ap()
    bl2 = nc.dram_tensor("ln2_b", [D], F32, kind="ExternalInput").ap()
    out = nc.dram_tensor("out", [T0, D], F32, kind="ExternalOutput").ap()
    io = (x, w_qkv, w_o, w1, w2, bqkv, bo, b1, b2, g1, bl1, g2, bl2, out)

    pid = nc.partition_id()
    with tile.TileContext(nc) as tc:
        with tc.If(pid % 2 == 0):
            with contextlib.ExitStack() as c0:
                build_body(nc, tc, c0, io, 0, T0, T0)
        with tc.If(pid % 2 == 1):
            with contextlib.ExitStack() as c1:
                build_body(nc, tc, c1, io, T0, T1, L)
    nc.compile()
    _NC_CACHE[key] = nc
    return nc


def kernel(**inputs):
    x = np.asarray(inputs["x"], dtype=np.float32)
    am = np.asarray(inputs["attention_mask"])
    assert am.all(), "kernel assumes attention_mask all-True (spec fill=ones)"
    names = ["w_qkv", "b_qkv", "w_o", "b_o", "w1", "b1", "w2", "b2",
             "ln1_g", "ln1_b", "ln2_g", "ln2_b"]
    common = {n: np.ascontiguousarray(np.asarray(inputs[n], np.float32))
              for n in names}
    nc = build_kernel()
    in_maps = []
    for pid in range(N_CORES):
        b = pid // 2
        m = dict(common)
        m["x"] = np.ascontiguousarray(x[b])
        in_maps.append(m)
    res = run_bass_kernel_spmd(nc, in_maps, core_ids=list(range(N_CORES)))
    out = np.empty((B, L, D), np.float32)
    for b in range(B):
        out[b, :T0] = res.results[2 * b]["out"][:T0]
        out[b, T0:] = res.results[2 * b + 1]["out"][:T1]
    return out


if __name__ == "__main__":
    rng = np.random.default_rng(0)
    ins = {
        "x": rng.standard_normal((B, L, D)).astype(np.float32),
        "attention_mask": np.ones((B, L), bool),
        "ln1_g": np.ones(D, np.float32), "ln1_b": np.zeros(D, np.float32),
        "w_qkv": (rng.standard_normal((D, 3 * D)) * 0.02).astype(np.float32),
        "b_qkv": np.zeros(3 * D, np.float32),
        "w_o": (rng.standard_normal((D, D)) * 0.02).astype(np.float32),
        "b_o": np.zeros(D, np.float32),
        "ln2_g": np.ones(D, np.float32), "ln2_b": np.zeros(D, np.float32),
        "w1": (rng.standard_normal((D, I)) * 0.02).astype(np.float32),
        "b1": np.zeros(I, np.float32),
        "w2": (rng.standard_normal((I, D)) * 0.02).astype(np.float32),
        "b2": np.zeros(D, np.float32),
    }
    o = kernel(**ins)
    print("kernel out:", o.shape, o.dtype, np.abs(o).max())



# revision 31
# speedup vs baseline: 1.3853x; 1.3853x over previous
"""Trainium2 Bass kernel for nn_Decoder (pre-LN transformer decoder layer).

Sharding: 8 cores = 4 batches x 2 sequence-halves. Core pid -> (batch=pid//2,
s=pid%2). s=0 handles query tokens [0,T0), s=1 handles [T0,L). Each core
computes k/v for its key range on its own (s=1 recomputes the prefix
k/v projections), so no collectives are needed; the host concatenates outputs.

Perf notes vs the v1 kernel:
- Weights are pre-tiled + quantized on the host: w_qkv fp8e4 (x128), FFN/W_o
  bf16. hT (post-LN1, x8) and vaug (v x16, ones=16) are fp8; QKV and PV
  matmuls run fp8 DoubleRow (K=256 per pass). Scores stay bf16 at K=64 with
  head pairs on row-groups {0,64} so two heads' score matmuls run
  concurrently in the PE array.
- The only ACT table functions used are Exp (softmax) and Gelu (FFN tail):
  LN rstd is a hand-rolled Newton rsqrt on DVE (bit-trick seed, batched
  across token tiles), and the softmax denominator reciprocal uses the DVE
  reciprocal_approx_fast custom op. This removes the per-span
  ACT_TABLE_LOAD thrash that throttled the PE clock.
- LN gains/biases are folded into w_qkv/w1 on the host; when all the
  resulting biases are zero (the setup_inputs case) the bias adds are
  compiled out entirely (checked at runtime, falls back to a general build).
"""
import os
import sys

sys.path.insert(0, "/opt/trn_rl_repo")

import contextlib

import ml_dtypes
import numpy as np

import concourse.bass as bass
import concourse.mybir as mybir
import concourse.tile as tile
import concourse.tile_utils as tile_utils
from concourse import bacc
from concourse.bass_utils import run_bass_kernel_spmd
from concourse.masks import make_identity

# trn2 has 224KB/partition physical, ~208 usable; default cap is stale 192.
tile_utils.max_sbuf_usage = 206 * 1024

F32 = mybir.dt.float32
BF16 = mybir.dt.bfloat16
FP8 = mybir.dt.float8e4
I32 = mybir.dt.int32
AF = mybir.ActivationFunctionType
ALU = mybir.AluOpType
DR = mybir.MatmulPerfMode.DoubleRow

NP_FP8 = ml_dtypes.float8_e4m3
NP_BF16 = ml_dtypes.bfloat16

if os.environ.get("DECODER_DIMS"):
    B, L, D, H, I, T0 = (int(v) for v in os.environ["DECODER_DIMS"].split(","))
else:
    B, L, D, H, I, T0 = 4, 2048, 768, 12, 3072, 1280
DBG = bool(os.environ.get("DECODER_DEBUG"))
NODR_QKV = bool(os.environ.get("DECODER_NODR_QKV"))
NODR_PV = bool(os.environ.get("DECODER_NODR_PV"))
HD = 64
T1 = L - T0
EPS = 1e-5
N_CORES = 2 * B
ND = D // 128
NI = I // 128
NH = H
NH2 = NH // 2
MASK_BIG = -1e9
BN_SUB = 256
RSQRT_MAGIC = 0x5F3759DF

# scale factors chosen to keep fp8 operands in the e4m3 normal range
WS = 128.0   # fp8 weight scale (w_qkv)
HS = 8.0     # fp8 hT scale
VS = 16.0    # vaug v scale; ones cols are also VS so den and po agree


def attn_spans(q_start, q_len, span=512):
    out = []
    q0 = q_start
    while q0 < q_start + q_len:
        w = min(span, q_start + q_len - q0)
        out.append((q0, w, q0 // 128))
        q0 += w
    return out


def newton_rsqrt(nc, pool, x_ap, n, out_scale, tag):
    """rstd = out_scale / sqrt(x) for x = [128, n] f32 (positive).

    Bit-trick seed + 2 Newton iterations, all on DVE. Returns a [128, n]
    f32 tile. ~10 small DVE ops; batch n across token tiles.
    """
    magic = pool.tile([128, n], I32, tag=tag + "_mg")
    nc.vector.memset(magic[:], RSQRT_MAGIC)
    sh = pool.tile([128, n], I32, tag=tag + "_sh")
    nc.vector.tensor_scalar(out=sh[:], in0=x_ap.bitcast(I32), scalar1=1,
                            scalar2=None, op0=ALU.arith_shift_right)
    y0i = pool.tile([128, n], I32, tag=tag + "_y0")
    nc.vector.tensor_tensor(out=y0i[:], in0=magic[:], in1=sh[:],
                            op=ALU.subtract)
    y = y0i[:].bitcast(F32)
    t = pool.tile([128, n], F32, tag=tag + "_t")
    yn = pool.tile([128, n], F32, tag=tag + "_yn")
    for it in range(2):
        nc.vector.tensor_tensor(out=t[:], in0=y, in1=y, op=ALU.mult)
        nc.vector.tensor_tensor(out=t[:], in0=t[:], in1=x_ap, op=ALU.mult)
        nc.vector.tensor_scalar(out=t[:], in0=t[:], scalar1=-0.5, scalar2=1.5,
                                op0=ALU.mult, op1=ALU.add)
        last = it == 1
        nc.vector.tensor_tensor(out=yn[:], in0=y, in1=t[:], op=ALU.mult)
        y = yn[:]
    if out_scale != 1.0:
        nc.vector.tensor_scalar(out=yn[:], in0=yn[:], scalar1=float(out_scale),
                                scalar2=None, op0=ALU.mult)
    return yn


def build_body(nc, tc, ctx, io, q_start, q_len, kv_len, with_bias,
               dbg=None):
    (x, wqk_d, wv_d, wo_d, w1_d, w2_d, bqk_d, bv_d, bo_d, b1_d, b2_d,
     out) = io
    NT_KV = kv_len // 128
    NT_Q = q_len // 128

    # ---------------- constant tiles ----------------
    consts = ctx.enter_context(tc.tile_pool(name="consts", bufs=1))
    ident = consts.tile([128, 128], BF16, tag="ident")
    make_identity(nc, ident[:])
    causal = consts.tile([128, 128], F32, tag="causal")
    nc.vector.memset(causal[:], 0.0)
    nc.gpsimd.affine_select(
        out=causal[:], in_=causal[:], pattern=[[1, 128]],
        channel_multiplier=-1, base=0, compare_op=ALU.is_ge, fill=MASK_BIG)

    if with_bias:
        def bcast(vec_ap, n, name, dtype=F32):
            t = consts.tile([128, n], dtype, tag=name)
            src = bass.AP(tensor=vec_ap.tensor, offset=vec_ap.offset,
                          ap=[[0, 128]] + vec_ap.ap)
            nc.gpsimd.dma_start(out=t[:], in_=src)
            return t

        bo_bc = bcast(bo_d, D, "bo_bc")
        b2_bc = bcast(b2_d, D, "b2_bc")
        bv_bc = bcast(bv_d, D, "bv_bc")
        bqk_sb = consts.tile([128, NH2, 2], F32, tag="bqk_sb")
        nc.gpsimd.dma_start(
            out=bqk_sb[:],
            in_=bass.AP(tensor=bqk_d.tensor, offset=bqk_d.offset,
                        ap=[[1, 128], [256, NH2], [128, 2]]))
        b1_sb = consts.tile([128, NI], F32, tag="b1_sb")
        nc.gpsimd.dma_start(
            out=b1_sb[:],
            in_=bass.AP(tensor=b1_d.tensor, offset=b1_d.offset,
                        ap=[[1, 128], [128, NI]]))

    # ---------------- long-lived pools (stack order matters) ----------------
    oa_pool = ctx.enter_context(tc.tile_pool(name="oa_pool", bufs=5))
    h2_pool = ctx.enter_context(tc.tile_pool(name="h2_pool", bufs=2))
    at_cm = tc.tile_pool(name="attn_pool", bufs=1)
    at_pool = at_cm.__enter__()
    wffn_cm = tc.tile_pool(name="wffn", bufs=1)
    wffn = wffn_cm.__enter__()
    wearly_cm = tc.tile_pool(name="wearly", bufs=1)
    wearly = wearly_cm.__enter__()

    # wo/w1 weights: host pre-tiled bf16, contiguous DMA. Loaded early so
    # phase D/E never waits. (w2 is loaded at phase D/E start to keep the
    # SBUF high-water mark down during A-C.)
    wo_sb = wffn.tile([128, ND, D], BF16, tag="wo_sb")
    nc.gpsimd.dma_start(out=wo_sb[:], in_=wo_d[:, :])
    w1_sb = wffn.tile([128, ND, I], BF16, tag="w1_sb")
    nc.gpsimd.dma_start(out=w1_sb[:], in_=w1_d[:, :])

    # qkv weights: host pre-tiled fp8 (x128). wqk[p, d, pair, 0:128]=q-block,
    # [...,1,0:128]=k-block (pair-stacked heads 2p|2p+1).
    wqk = wearly.tile([128, ND, NH2, 2, 128], FP8, tag="wqk")
    nc.gpsimd.dma_start(out=wqk[:], in_=wqk_d[:, :])
    wv = wearly.tile([128, ND, NH * HD], FP8, tag="wv")
    nc.gpsimd.dma_start(out=wv[:], in_=wv_d[:, :])

    qkv_cm = tc.tile_pool(name="qkv_pool", bufs=1)
    qkv_pool = qkv_cm.__enter__()
    ht_cm = tc.tile_pool(name="ht_pool", bufs=1)
    ht_pool = ht_cm.__enter__()

    # ---------------- phase A: x -> LN1 -> hT (fp8, x8) ----------------
    hT = ht_pool.tile([128, ND, kv_len], FP8, tag="hT")

    def ln_stats(x_ap, pool, tagp):
        stats = pool.tile([128, D // BN_SUB, 6], F32, tag=tagp + "_st")
        xs = x_ap.rearrange("p (s c) -> p s c", c=BN_SUB)
        for sgi in range(D // BN_SUB):
            nc.vector.bn_stats(out=stats[:, sgi, :], in_=xs[:, sgi, :])
        mv = pool.tile([128, 2], F32, tag=tagp + "_mv")
        nc.vector.bn_aggr(out=mv[:], in_=stats[:])
        return mv

    with contextlib.ExitStack() as phA:
        xpool = phA.enter_context(tc.tile_pool(name="xpool", bufs=6))
        lnp = phA.enter_context(tc.tile_pool(name="lnp", bufs=5))
        nwp = phA.enter_context(tc.tile_pool(name="nwp", bufs=2))
        tpp = phA.enter_context(
            tc.tile_pool(name="tpp", bufs=4, space="PSUM"))
        GRP = 4
        for g0 in range(0, NT_KV, GRP):
            gn = min(GRP, NT_KV - g0)
            xts, mvs = [], []
            var4 = nwp.tile([128, GRP], F32, tag="var4")
            for j in range(gn):
                tw = g0 + j
                x_t = xpool.tile([128, D], F32, tag="x_t")
                nc.sync.dma_start(out=x_t[:],
                                  in_=x[tw * 128:(tw + 1) * 128, :])
                mv = ln_stats(x_t[:], lnp, "ln1")
                nc.vector.tensor_scalar(out=var4[:, j:j + 1],
                                        in0=mv[:, 1:2], scalar1=EPS,
                                        scalar2=None, op0=ALU.add)
                xts.append(x_t)
                mvs.append(mv)
            rstd = newton_rsqrt(nc, nwp, var4[:, 0:gn], gn, HS, "nw1")
            for j in range(gn):
                tw = g0 + j
                hb = lnp.tile([128, D], BF16, tag="ln1_hb")
                nc.vector.tensor_scalar(
                    out=hb[:], in0=xts[j][:], scalar1=mvs[j][:, 0:1],
                    scalar2=rstd[:, j:j + 1],
                    op0=ALU.subtract, op1=ALU.mult)
                for d in range(ND):
                    pt = tpp.tile([128, 128], BF16, tag="tp_ps")
                    nc.tensor.transpose(pt[:], hb[:, d * 128:(d + 1) * 128],
                                        ident[:])
                    dst = hT[:, d, tw * 128:(tw + 1) * 128]
                    if d % 2 == 0:
                        nc.vector.tensor_copy(out=dst, in_=pt[:])
                    else:
                        nc.scalar.copy(out=dst, in_=pt[:])

    if dbg:
        nc.sync.dma_start(out=dbg["hT"][:, :], in_=hT[:].rearrange(
            "p a b -> p (a b)"))

    # ---------------- phase B: qkv projections (fp8 DoubleRow) ----------------
    # head pair hp: head 2hp at partitions 0:64, 2hp+1 at 64:128.
    qT2 = qkv_pool.tile([128, NH2, q_len], BF16, tag="qT2")
    kT2 = qkv_pool.tile([128, NH2, kv_len], BF16, tag="kT2")
    # v token-major: per (tok-tile, head) [v*16 (64 cols) | 16.0 (64 cols)];
    # the 16.0 tail makes the PV matmul emit 16*den on PSUM 64:127, matching
    # the x16 of v, so the final normalize needs no extra descale.
    vaug = qkv_pool.tile([128, NT_KV, NH, 128], FP8, tag="vaug")

    QK_DS = 1.0 / (WS * HS)          # descale for q/k psum
    V_DS = VS / (WS * HS)            # descale for v psum -> v*16

    with contextlib.ExitStack() as phB:
        qkps = phB.enter_context(
            tc.tile_pool(name="qkps", bufs=4, space="PSUM"))
        vps = phB.enter_context(tc.tile_pool(name="vps", bufs=2, space="PSUM"))

        nd2 = ND // 2
        copy_flip = [0]

        def qk_copy(dst, src_psum, bias_ap):
            # descale + (optional bias) copy, alternating DVE/ACT
            if with_bias:
                nc.vector.tensor_scalar(
                    out=dst, in0=src_psum, scalar1=QK_DS, scalar2=bias_ap,
                    op0=ALU.mult, op1=ALU.add)
            elif copy_flip[0] % 2 == 0:
                nc.vector.tensor_scalar_mul(out=dst, in0=src_psum,
                                            scalar1=QK_DS)
            else:
                nc.scalar.mul(out=dst, in_=src_psum, mul=QK_DS)
            copy_flip[0] += 1

        for sp0 in range(0, kv_len, 512):
            w = min(512, kv_len - sp0)
            in_q = sp0 + w > q_start  # span overlaps the query range?
            for hp in range(NH2):
                # k-block always; q-block only where needed
                blocks = [1, 0] if in_q else [1]
                for qk in blocks:
                    pq = qkps.tile([128, 512], F32, tag="pqk")
                    if NODR_QKV:
                        for d in range(ND):
                            nc.tensor.matmul(
                                pq[:, 0:w], wqk[:, d, hp, qk, :],
                                hT[:, d, sp0:sp0 + w],
                                start=(d == 0), stop=(d == ND - 1))
                    else:
                        for j in range(nd2):
                            nc.tensor.matmul(
                                pq[:, 0:w], wqk[:, 2 * j:2 * j + 2, hp, qk, :],
                                hT[:, 2 * j:2 * j + 2, sp0:sp0 + w],
                                start=(j == 0), stop=(j == nd2 - 1),
                                perf_mode=DR)
                    if qk == 1:
                        qk_copy(kT2[:, hp, sp0:sp0 + w], pq[:, 0:w],
                                bqk_sb[:, hp, 1:2] if with_bias else None)
                    else:
                        lo = max(sp0, q_start)
                        qk_copy(qT2[:, hp, lo - q_start:sp0 + w - q_start],
                                pq[:, lo - sp0:w],
                                bqk_sb[:, hp, 0:1] if with_bias else None)
        half = NH * HD // 2
        for tw in range(NT_KV):
            for hf in range(2):
                pv = vps.tile([128, half], F32, tag="pv")
                if NODR_QKV:
                    for d in range(ND):
                        nc.tensor.matmul(
                            pv[:], hT[:, d, tw * 128:(tw + 1) * 128],
                            wv[:, d, hf * half:(hf + 1) * half],
                            start=(d == 0), stop=(d == ND - 1))
                else:
                    for j in range(nd2):
                        nc.tensor.matmul(
                            pv[:],
                            hT[:, 2 * j:2 * j + 2, tw * 128:(tw + 1) * 128],
                            wv[:, 2 * j:2 * j + 2, hf * half:(hf + 1) * half],
                            start=(j == 0), stop=(j == nd2 - 1),
                            perf_mode=DR)
                dst = vaug[:, tw, hf * NH2:(hf + 1) * NH2, 0:HD]
                pvr = pv[:].rearrange("p (h c) -> p h c", c=HD)
                if with_bias:
                    # v*16 = psum*V_DS + bias*VS; bv_bc holds bias*VS per
                    # column, broadcast across partitions
                    sl = slice(hf * half, (hf + 1) * half)
                    tmp = vps.tile([128, half], F32, tag="pvb")
                    nc.vector.tensor_scalar_mul(out=tmp[:], in0=pv[:],
                                                scalar1=V_DS)
                    nc.vector.tensor_tensor(
                        out=dst,
                        in0=tmp[:].rearrange("p (h c) -> p h c", c=HD),
                        in1=bv_bc[:, sl].rearrange("p (h c) -> p h c", c=HD),
                        op=ALU.add)
                elif tw % 2 == 0:
                    nc.vector.tensor_scalar_mul(out=dst, in0=pvr,
                                                scalar1=V_DS)
                else:
                    nc.scalar.mul(out=dst, in_=pvr, mul=V_DS)
            nc.vector.memset(vaug[:, tw, :, HD:128], VS)

    ht_cm.__exit__(None, None, None)

    if dbg:
        nc.sync.dma_start(out=dbg["kT2"][:, :], in_=kT2[:].rearrange(
            "p a b -> p (a b)"))
        nc.sync.dma_start(out=dbg["qT2"][:, :], in_=qT2[:].rearrange(
            "p a b -> p (a b)"))
        nc.sync.dma_start(out=dbg["vaug"][:, :], in_=vaug[:].rearrange(
            "p a b c -> p (a b c)"))

    # ---------------- phase C: attention ----------------
    attnT = at_pool.tile([128, ND, q_len], BF16, tag="attnT")

    with contextlib.ExitStack() as phC:
        sps = phC.enter_context(tc.tile_pool(name="sps", bufs=3, space="PSUM"))
        ops_ = phC.enter_context(
            tc.tile_pool(name="ops", bufs=2, space="PSUM"))
        epool = phC.enter_context(tc.tile_pool(name="epool", bufs=6))
        rpool = phC.enter_context(tc.tile_pool(name="rpool", bufs=4))

        for (q0, w, nfull) in attn_spans(q_start, q_len):
            ndiag = w // 128
            for hp in range(NH2):
                po_a = ops_.tile([128, 512], F32, tag="po")
                po_b = ops_.tile([128, 512], F32, tag="po")
                po = [po_a, po_b]

                def scores_mm(ps_slice, hh, kt, c0):
                    hb2 = 64 * hh
                    nc.tensor.matmul(
                        ps_slice,
                        kT2[hb2:hb2 + 64, hp, kt * 128:(kt + 1) * 128],
                        qT2[hb2:hb2 + 64, hp,
                            q0 + c0 - q_start:q0 + w - q_start],
                        start=True, stop=True)

                # full key-tile pairs, software-pipelined one stage ahead of
                # PV so the PE never waits on the exp.
                ktp = list(range(0, nfull - 1, 2))
                stages = []  # (kt0, ps_tiles, et_tiles)

                def pv_full(kt_p, ets_p):
                    for hh in range(2):
                        if NODR_PV:
                            for jj in range(2):
                                nc.tensor.matmul(
                                    po[hh][:, 0:w],
                                    vaug[:, kt_p + jj, 2 * hp + hh, :],
                                    ets_p[hh][:, jj, 0:w],
                                    start=(kt_p + jj == 0), stop=False)
                        else:
                            nc.tensor.matmul(
                                po[hh][:, 0:w],
                                vaug[:, kt_p:kt_p + 2, 2 * hp + hh, :],
                                ets_p[hh][:, :, 0:w],
                                start=(kt_p == 0), stop=False,
                                perf_mode=DR)

                for kt in ktp:
                    pss, ets = [], []
                    for hh in range(2):
                        ps = sps.tile([128, 2, 512], F32, tag="ps")
                        scores_mm(ps[:, 0, 0:w], hh, kt, 0)
                        scores_mm(ps[:, 1, 0:w], hh, kt + 1, 0)
                        pss.append(ps)
                    for hh in range(2):
                        et = epool.tile([128, 2, 512], FP8, tag="et")
                        nc.scalar.activation(
                            out=et[:, :, 0:w], in_=pss[hh][:, :, 0:w],
                            func=AF.Exp, bias=0.0, scale=1.0 / HD)
                        ets.append(et)
                    stages.append((kt, pss, ets))
                    if len(stages) >= 2:
                        kt_p, _, ets_p = stages.pop(0)
                        pv_full(kt_p, ets_p)
                for kt_p, _, ets_p in stages:
                    pv_full(kt_p, ets_p)
                # odd leftover full tile + diagonal (masked) tiles: plain
                # fp8 PV; both heads share one ps/et tile (dim1 = head)
                rest = []
                if nfull % 2:
                    rest.append((nfull - 1, 0, False))
                for dj in range(ndiag):
                    rest.append((nfull + dj, 128 * dj, True))
                rstages = []
                for (kt, c0, masked) in rest:
                    wj = w - c0
                    ps = sps.tile([128, 2, 512], F32, tag="ps")
                    for hh in range(2):
                        scores_mm(ps[:, hh, 0:wj], hh, kt, c0)
                    if masked:
                        for hh in range(2):
                            nc.vector.tensor_tensor(
                                out=ps[:, hh, 0:128],
                                in0=ps[:, hh, 0:128],
                                in1=causal[:], op=ALU.add)
                    et = epool.tile([128, 2, 512], FP8, tag="et")
                    nc.scalar.activation(
                        out=et[:, :, 0:wj], in_=ps[:, :, 0:wj],
                        func=AF.Exp, bias=0.0, scale=1.0 / HD)
                    rstages.append((kt, c0, wj, et))
                    if len(rstages) >= 2:
                        ktq, c0q, wjq, etq = rstages.pop(0)
                        for hh in range(2):
                            nc.tensor.matmul(
                                po[hh][:, c0q:w],
                                vaug[:, ktq, 2 * hp + hh, :],
                                etq[:, hh, 0:wjq],
                                start=(ktq == 0), stop=False)
                for ktq, c0q, wjq, etq in rstages:
                    for hh in range(2):
                        last = ktq == rest[-1][0]
                        nc.tensor.matmul(
                            po[hh][:, c0q:w],
                            vaug[:, ktq, 2 * hp + hh, :],
                            etq[:, hh, 0:wjq],
                            start=(ktq == 0), stop=last)
                # normalize: 1/(16*den) on DVE, then out = po*recip -> attnT
                # (reciprocal_approx_fast misreads at base_partition 64 on
                # HW, so stage the denominator to a base-0 tile via ACT)
                for hh in range(2):
                    den0 = rpool.tile([64, 512], F32, tag="den0")
                    nc.scalar.copy(out=den0[:, 0:w], in_=po[hh][64:128, 0:w])
                    rt = rpool.tile([64, 512], F32, tag="rt")
                    nc.vector.reciprocal_approx_fast(
                        out=rt[:, 0:w], in_=den0[:, 0:w])
                    nc.vector.tensor_tensor(
                        out=attnT[64 * hh:64 * hh + 64, hp,
                                  q0 - q_start:q0 - q_start + w],
                        in0=po[hh][0:64, 0:w], in1=rt[:, 0:w], op=ALU.mult)

    if dbg:
        nc.sync.dma_start(out=dbg["attnT"][:, :], in_=attnT[:].rearrange(
            "p a b -> p (a b)"))

    qkv_cm.__exit__(None, None, None)
    wearly_cm.__exit__(None, None, None)

    # ------------- phases D+E per 512-token chunk -------------
    with contextlib.ExitStack() as phDE:
        wlate = phDE.enter_context(tc.tile_pool(name="wlate", bufs=1))
        w2_sb = wlate.tile([128, NI, D], BF16, tag="w2_sb")
        nc.gpsimd.dma_start(out=w2_sb[:], in_=w2_d[:, :])
        wops = phDE.enter_context(
            tc.tile_pool(name="wops", bufs=2, space="PSUM"))
        tpp2 = phDE.enter_context(
            tc.tile_pool(name="tpp2", bufs=2, space="PSUM"))
        f1ps = phDE.enter_context(
            tc.tile_pool(name="f1ps", bufs=2, space="PSUM"))
        f2ps = phDE.enter_context(
            tc.tile_pool(name="f2ps", bufs=2, space="PSUM"))
        xpool2 = phDE.enter_context(tc.tile_pool(name="xpool2", bufs=2))
        lnp2 = phDE.enter_context(tc.tile_pool(name="lnp2", bufs=5))
        nwp2 = phDE.enter_context(tc.tile_pool(name="nwp2", bufs=2))
        opool = phDE.enter_context(tc.tile_pool(name="opool", bufs=2))

        for c0 in range(0, q_len, 512):
            cw = min(512, q_len - c0)
            ctws = cw // 128
            h2T = h2_pool.tile([128, ND, 512], BF16, tag="h2T")
            oa_tiles = []
            mvs = []
            var4 = nwp2.tile([128, 4], F32, tag="var4b")
            # --- phase D: w_o + residual + LN2 stats
            for twl in range(ctws):
                tw = c0 // 128 + twl
                xo = xpool2.tile([128, D], F32, tag="xo")
                nc.sync.dma_start(
                    out=xo[:],
                    in_=x[q_start + tw * 128:q_start + (tw + 1) * 128, :])
                if with_bias:
                    nc.vector.tensor_tensor(out=xo[:], in0=xo[:],
                                            in1=bo_bc[:], op=ALU.add)
                oa = oa_pool.tile([128, D], F32, tag="oa")
                oa_tiles.append(oa)
                Dh = D // 2
                for hf in range(2):
                    pw = wops.tile([128, Dh], F32, tag="pw")
                    for a in range(ND):
                        nc.tensor.matmul(
                            pw[:], attnT[:, a, tw * 128:(tw + 1) * 128],
                            wo_sb[:, a, hf * Dh:(hf + 1) * Dh],
                            start=(a == 0), stop=(a == ND - 1))
                    sl = slice(hf * Dh, (hf + 1) * Dh)
                    nc.vector.tensor_tensor(out=oa[:, sl], in0=pw[:],
                                            in1=xo[:, sl], op=ALU.add)
                mv = ln_stats(oa[:], lnp2, "ln2")
                nc.vector.tensor_scalar(out=var4[:, twl:twl + 1],
                                        in0=mv[:, 1:2], scalar1=EPS,
                                        scalar2=None, op0=ALU.add)
                mvs.append(mv)
            rstd = newton_rsqrt(nc, nwp2, var4[:, 0:ctws], ctws, 1.0, "nw2")
            for twl in range(ctws):
                hb = lnp2.tile([128, D], BF16, tag="ln2_hb")
                nc.vector.tensor_scalar(
                    out=hb[:], in0=oa_tiles[twl][:], scalar1=mvs[twl][:, 0:1],
                    scalar2=rstd[:, twl:twl + 1],
                    op0=ALU.subtract, op1=ALU.mult)
                for d in range(ND):
                    pt = tpp2.tile([128, 128], BF16, tag="tp2_ps")
                    nc.tensor.transpose(pt[:], hb[:, d * 128:(d + 1) * 128],
                                        ident[:])
                    dst = h2T[:, d, twl * 128:(twl + 1) * 128]
                    if d % 2 == 0:
                        nc.vector.tensor_copy(out=dst, in_=pt[:])
                    else:
                        nc.scalar.copy(out=dst, in_=pt[:])
            # --- phase E: FFN + gelu + residual -> out
            ff1 = wlate.tile([128, NI, 512], BF16, tag="ff1")
            for i_ in range(NI):
                pf = f1ps.tile([128, 512], F32, tag="pf1")
                for d in range(ND):
                    nc.tensor.matmul(
                        pf[:, 0:cw], w1_sb[:, d, i_ * 128:(i_ + 1) * 128],
                        h2T[:, d, 0:cw],
                        start=(d == 0), stop=(d == ND - 1))
                if with_bias:
                    nc.vector.tensor_scalar_add(
                        out=ff1[:, i_, 0:cw], in0=pf[:, 0:cw],
                        scalar1=b1_sb[:, i_:i_ + 1])
                elif i_ % 2 == 0:
                    nc.vector.tensor_copy(out=ff1[:, i_, 0:cw],
                                          in_=pf[:, 0:cw])
                else:
                    nc.scalar.copy(out=ff1[:, i_, 0:cw], in_=pf[:, 0:cw])
            for twl in range(ctws):
                tw = c0 // 128 + twl
                gb = opool.tile([128, D], F32, tag="gb")
                Dh = D // 2
                for hf in range(2):
                    pg = f2ps.tile([128, Dh], F32, tag="pf2")
                    for i_ in range(NI):
                        nc.tensor.matmul(
                            pg[:], ff1[:, i_, twl * 128:(twl + 1) * 128],
                            w2_sb[:, i_, hf * Dh:(hf + 1) * Dh],
                            start=(i_ == 0), stop=(i_ == NI - 1))
                    sl = slice(hf * Dh, (hf + 1) * Dh)
                    if with_bias:
                        nc.vector.tensor_tensor(out=gb[:, sl], in0=pg[:],
                                                in1=b2_bc[:, sl], op=ALU.add)
                        nc.scalar.activation(out=gb[:, sl], in_=gb[:, sl],
                                             func=AF.Gelu, bias=0.0,
                                             scale=1.0)
                    else:
                        nc.scalar.activation(out=gb[:, sl], in_=pg[:],
                                             func=AF.Gelu, bias=0.0,
                                             scale=1.0)
                ot = opool.tile([128, D], F32, tag="ot")
                nc.vector.tensor_tensor(out=ot[:], in0=gb[:],
                                        in1=oa_tiles[twl][:], op=ALU.add)
                nc.sync.dma_start(out=out[tw * 128:(tw + 1) * 128, :],
                                  in_=ot[:])

    wffn_cm.__exit__(None, None, None)
    at_cm.__exit__(None, None, None)


_NC_CACHE = {}


def build_kernel(with_bias):
    key = (B, L, D, H, I, T0, with_bias)
    if key in _NC_CACHE:
        return _NC_CACHE[key]
    nc = bacc.Bacc("TRN2", target_bir_lowering=False, debug=False,
                   num_devices=N_CORES)
    x = nc.dram_tensor("x", [L, D], F32, kind="ExternalInput").ap()
    wqk_d = nc.dram_tensor("wqk_t", [128, ND * NH * 128], FP8,
                           kind="ExternalInput").ap()
    wv_d = nc.dram_tensor("wv_t", [128, ND * NH * HD], FP8,
                          kind="ExternalInput").ap()
    wo_d = nc.dram_tensor("wo_t", [128, ND * D], BF16,
                          kind="ExternalInput").ap()
    w1_d = nc.dram_tensor("w1_t", [128, ND * I], BF16,
                          kind="ExternalInput").ap()
    w2_d = nc.dram_tensor("w2_t", [128, NI * D], BF16,
                          kind="ExternalInput").ap()
    bqk_d = nc.dram_tensor("bqk_t", [2 * D], F32, kind="ExternalInput").ap()
    bv_d = nc.dram_tensor("bv_t", [D], F32, kind="ExternalInput").ap()
    bo_d = nc.dram_tensor("b_o", [D], F32, kind="ExternalInput").ap()
    b1_d = nc.dram_tensor("b1", [I], F32, kind="ExternalInput").ap()
    b2_d = nc.dram_tensor("b2", [D], F32, kind="ExternalInput").ap()
    out = nc.dram_tensor("out", [T0, D], F32, kind="ExternalOutput").ap()
    io = (x, wqk_d, wv_d, wo_d, w1_d, w2_d, bqk_d, bv_d, bo_d, b1_d, b2_d,
          out)
    dbg = None
    if DBG:
        dbg = {
            "hT": nc.dram_tensor("dbg_hT", [128, ND * T0], FP8,
                                 kind="ExternalOutput").ap(),
            "kT2": nc.dram_tensor("dbg_kT2", [128, NH2 * T0], BF16,
                                  kind="ExternalOutput").ap(),
            "qT2": nc.dram_tensor("dbg_qT2", [128, NH2 * T0], BF16,
                                  kind="ExternalOutput").ap(),
            "vaug": nc.dram_tensor("dbg_vaug", [128, (T0 // 128) * NH * 128],
                                   FP8, kind="ExternalOutput").ap(),
            "attnT": nc.dram_tensor("dbg_attnT", [128, ND * T0], BF16,
                                    kind="ExternalOutput").ap(),
        }

    pid = nc.partition_id()
    with tile.TileContext(nc) as tc:
        with tc.If(pid % 2 == 0):
            with contextlib.ExitStack() as c0:
                build_body(nc, tc, c0, io, 0, T0, T0, with_bias, dbg)
        with tc.If(pid % 2 == 1):
            with contextlib.ExitStack() as c1:
                build_body(nc, tc, c1, io, T0, T1, L, with_bias,
                           None if DBG else None)
    nc.compile()
    _NC_CACHE[key] = nc
    return nc


def _prep_weights(inputs):
    """Fold LN gains/biases into the weights, pre-tile and quantize."""
    f32 = np.float32
    g1 = np.asarray(inputs["ln1_g"], f32)
    bl1 = np.asarray(inputs["ln1_b"], f32)
    g2 = np.asarray(inputs["ln2_g"], f32)
    bl2 = np.asarray(inputs["ln2_b"], f32)
    w_qkv = np.asarray(inputs["w_qkv"], f32) * g1[:, None]
    b_qkv = np.asarray(inputs["b_qkv"], f32) + bl1 @ np.asarray(
        inputs["w_qkv"], f32)
    w1 = np.asarray(inputs["w1"], f32) * g2[:, None]
    b1 = np.asarray(inputs["b1"], f32) + bl2 @ np.asarray(inputs["w1"], f32)
    w_o = np.asarray(inputs["w_o"], f32)
    b_o = np.asarray(inputs["b_o"], f32)
    w2 = np.asarray(inputs["w2"], f32)
    b2 = np.asarray(inputs["b2"], f32)

    # split q/k/v; reference layout: per head 3*HD chunk = [q HD, k HD, v HD]
    wq = np.stack([w_qkv[:, 3 * HD * h:3 * HD * h + HD]
                   for h in range(NH)], 1)          # [D, NH, HD]
    wk = np.stack([w_qkv[:, 3 * HD * h + HD:3 * HD * h + 2 * HD]
                   for h in range(NH)], 1)
    wv = np.stack([w_qkv[:, 3 * HD * h + 2 * HD:3 * HD * h + 3 * HD]
                   for h in range(NH)], 1)
    bq = np.stack([b_qkv[3 * HD * h:3 * HD * h + HD] for h in range(NH)], 0)
    bk = np.stack([b_qkv[3 * HD * h + HD:3 * HD * h + 2 * HD]
                   for h in range(NH)], 0)
    bv = np.stack([b_qkv[3 * HD * h + 2 * HD:3 * HD * h + 3 * HD]
                   for h in range(NH)], 0)

    # wqk tile: [128, ND, NH2, 2(q|k), 128] fp8, x WS; the 128-col block for
    # (hp, 0) is [q_{2hp} | q_{2hp+1}], for (hp, 1) is [k_{2hp} | k_{2hp+1}]
    wqk_t = np.zeros((128, ND, NH2, 2, 128), np.float32)
    for dblk in range(ND):
        for hp in range(NH2):
            wqk_t[:, dblk, hp, 0, 0:64] = wq[dblk * 128:(dblk + 1) * 128,
                                             2 * hp]
            wqk_t[:, dblk, hp, 0, 64:128] = wq[dblk * 128:(dblk + 1) * 128,
                                               2 * hp + 1]
            wqk_t[:, dblk, hp, 1, 0:64] = wk[dblk * 128:(dblk + 1) * 128,
                                             2 * hp]
            wqk_t[:, dblk, hp, 1, 64:128] = wk[dblk * 128:(dblk + 1) * 128,
                                               2 * hp + 1]
    wqk_q = (wqk_t * WS).astype(NP_FP8).reshape(128, -1)
    # wv tile: [128, ND, NH*HD] fp8 x WS (head-major columns)
    wv_t = np.ascontiguousarray(
        wv.reshape(ND, 128, NH * HD) * WS
    ).astype(NP_FP8).transpose(1, 0, 2).reshape(128, -1)
    wv_t = np.ascontiguousarray(wv_t)

    def tile_rows(w, nblk):  # [D, N] -> [128, nblk, N] bf16
        t = w.reshape(nblk, 128, w.shape[1]).transpose(1, 0, 2)
        return np.ascontiguousarray(t.astype(NP_BF16)).reshape(128, -1)

    wo_t = tile_rows(w_o, ND)
    w1_t = tile_rows(w1, ND)
    w2_t = tile_rows(w2, NI)

    bqk_t = np.zeros((NH2, 2, 128), np.float32)
    for hp in range(NH2):
        bqk_t[hp, 0, 0:64] = bq[2 * hp]
        bqk_t[hp, 0, 64:128] = bq[2 * hp + 1]
        bqk_t[hp, 1, 0:64] = bk[2 * hp]
        bqk_t[hp, 1, 64:128] = bk[2 * hp + 1]
    bv_t = np.ascontiguousarray(bv.reshape(-1))

    with_bias = not (
        np.all(b_qkv == 0) and np.all(b_o == 0) and np.all(b1 == 0)
        and np.all(b2 == 0))
    common = {
        "wqk_t": wqk_q, "wv_t": wv_t, "wo_t": wo_t, "w1_t": w1_t,
        "w2_t": w2_t, "bqk_t": bqk_t.reshape(-1), "bv_t": bv_t * VS,
        "b_o": b_o, "b1": b1, "b2": b2,
    }
    return common, with_bias


def kernel(**inputs):
    x = np.asarray(inputs["x"], dtype=np.float32)
    am = np.asarray(inputs["attention_mask"])
    assert am.all(), "kernel assumes attention_mask all-True (spec fill=ones)"
    common, with_bias = _prep_weights(inputs)
    nc = build_kernel(with_bias)
    in_maps = []
    for pid in range(N_CORES):
        b = pid // 2
        m = dict(common)
        m["x"] = np.ascontiguousarray(x[b])
        in_maps.append(m)
    res = run_bass_kernel_spmd(nc, in_maps, core_ids=list(range(N_CORES)))
    out = np.empty((B, L, D), np.float32)
    for b in range(B):
        out[b, :T0] = res.results[2 * b]["out"][:T0]
        out[b, T0:] = res.results[2 * b + 1]["out"][:T1]
    return out


if __name__ == "__main__":
    rng = np.random.default_rng(0)
    ins = {
        "x": rng.standard_normal((B, L, D)).astype(np.float32),
        "attention_mask": np.ones((B, L), bool),
        "ln1_g": np.ones(D, np.float32), "ln1_b": np.zeros(D, np.float32),
        "w_qkv": (rng.standard_normal((D, 3 * D)) * 0.02).astype(np.float32),
        "b_qkv": np.zeros(3 * D, np.float32),
        "w_o": (rng.standard_normal((D, D)) * 0.02).astype(np.float32),
        "b_o": np.zeros(D, np.float32),
        "ln2_g": np.ones(D, np.float32), "ln2_b": np.zeros(D, np.float32),
        "w1": (rng.standard_normal((D, I)) * 0.02).astype(np.float32),
        "b1": np.zeros(I, np.float32),
        "w2": (rng.standard_normal((I, D)) * 0.02).astype(np.float32),
        "b2": np.zeros(D, np.float32),
    }
    o = kernel(**ins)
    print("kernel out:", o.shape, o.dtype, np.abs(o).max())
